# revision 49
# baseline (speedup 1.0000x reference)
"""MultiHeadAttention Trainium2 kernel, 8-core SPMD, v3 (A2).

Sharding: core = (batch b, head-group g), b in {0,1}, g in {0..3}.
Each core computes 4 heads of one batch (tensor parallel on heads,
data parallel on batch). Out-projection partials are summed on host
(bias also added on host).

vs the original baseline:
- all matmul operands bf16 (same PE rate as fp32r, no small-free
  penalty on the diagonal tiles)
- xT resident in SBUF (loaded once as bf16) -> V/K projections don't
  re-stream it; projections are never DMA-bound
- attention layout [k, q]: scores^T in PSUM, exp to SBUF, PV and
  rowsum (ones-matmul) accumulate per k-tile with no row barrier;
  causal mask via post-exp 0/1 multiply on the diagonal tiles
- DRAM inputs pre-packed host-side into SBUF layout (wide DMA lines)
- one whole-kernel SBUF pool with stable per-tag slots, so in a
  repeated body each rep's input loads overlap the previous rep's
  attention phase (WAR lands on that slot's own early consumers)
- out-projection interleaved into the ACT-bound attention phase as PE
  filler; output bf16, bias + partial sums on host

Self-contained: hardcodes shapes B=2, S=2048, D=2048, H=16.
"""

import ml_dtypes
import numpy as np

import concourse.bacc as bacc
import concourse.mybir as mybir
import concourse.tile as tile
from concourse.bass_utils import run_bass_kernel_spmd

B, S, D = 2, 2048, 2048
H = 16
HD = D // H          # 128 head dim
G = 4                # head groups (tensor parallel degree)
HPG = H // G         # 4 heads per group
DG = HPG * HD        # 512 features per group
NCORES = 8
NTC = D // 128       # 16 contraction chunks
NQT = S // 128       # 16 q tiles of 128
NSC = S // 512       # 4 seq chunks of 512
SCALE = float(1.0 / np.sqrt(np.float32(S)))
NEG = -1.0e9

F32 = mybir.dt.float32
BF16 = mybir.dt.bfloat16
EXP = mybir.ActivationFunctionType.Exp

_CACHE = {}


def _outproj(nc, patt, etp, ct, wo_sb, y, ic):
    """Out-projection for q chunk ic: y[ic*512:(ic+1)*512, :]."""
    for it in range(ic * 4, ic * 4 + 4):
        t0 = it * 128
        ysb = etp.tile([128, D], BF16, tag="ysb", name="ysb", bufs=2)
        for oc in range(4):
            o0 = oc * 512
            yps = patt.tile([128, 512], F32, tag="ctx", name="yps", bufs=3)
            for h in range(4):
                nc.tensor.matmul(
                    yps[:],
                    ct[h][:, t0 : t0 + 128],
                    wo_sb[:, h * D + o0 : h * D + o0 + 512],
                    start=(h == 0), stop=(h == 3),
                )
            nc.vector.tensor_copy(ysb[:, o0 : o0 + 512], yps[:])
        nc.scalar.dma_start(y[t0 : t0 + 128, :], ysb[:])


def _build(nreps=1, trace_sim=False, debug=0):
    nc = bacc.Bacc(target_bir_lowering=False, trn_type="TRN2")
    xT = nc.dram_tensor("xT", [128, NTC * S], BF16, kind="ExternalInput")
    wqT = nc.dram_tensor("wqT", [128, NTC * DG], BF16, kind="ExternalInput")
    wkT = nc.dram_tensor("wkT", [128, NTC * DG], BF16, kind="ExternalInput")
    wvT = nc.dram_tensor("wvT", [128, NTC * DG], BF16, kind="ExternalInput")
    woT = nc.dram_tensor("woT", [128, HPG * D], BF16, kind="ExternalInput")
    mask = nc.dram_tensor("mask", [128, 128], F32, kind="ExternalInput")
    y = nc.dram_tensor("y", [S, D], BF16, kind="ExternalOutput")

    with tile.TileContext(nc, trace_sim=trace_sim) as tc:
      # one SBUF pool for the whole kernel: tags give every rep identical
      # slots, so a rep's input loads only WAR-wait on that slot's own
      # consumers in the previous rep (wq: QK end; xT: V end), overlapping
      # the previous rep's attention phase.
      with tc.tile_pool(name="res", bufs=1) as res:
        for _rep in range(nreps):
            # ---- resident tiles ----
            xt = res.tile([128, NTC * S], BF16, tag="xt", name="xt")          # 64KB
            wv_sb = res.tile([128, NTC * DG], BF16, tag="wv", name="wv")      # 16KB
            qt = [res.tile([128, S], BF16, tag=f"qt{h}", name=f"qt{h}") for h in range(HPG)]
            kt = [res.tile([128, S], BF16, tag=f"kt{h}", name=f"kt{h}") for h in range(HPG)]
            vt = res.tile([128, NTC * DG], BF16, tag="vt", name="vt")         # 16KB
            ct = [res.tile([128, S], BF16, tag=f"ct{h}", name=f"ct{h}") for h in range(HPG)]
            mask_t = res.tile([128, 128], F32, tag="mask", name="mask_t")
            onesb = res.tile([128, 128], BF16, tag="onesb", name="onesb")
            nc.vector.memset(onesb[:], 1.0)

            # ---- phase QK: Q and K projections (xt resident) ----
            if True:
                wp = res
                wq_sb = wp.tile([128, NTC * DG], BF16, tag="wq", name="wq")
                wk_sb = wp.tile([128, NTC * DG], BF16, tag="wk", name="wk")
                # first matmuls need wq quarter 0 + xt quarter 0 of chunk 0:
                # issue those first, split across the two HWDGE queues.
                quarters = ((0, 1), (1, 1), (2, 2), (4, 4), (8, 4), (12, 4))
                for c0, nch in quarters:
                    nc.sync.dma_start(
                        wq_sb[:, c0 * DG : (c0 + nch) * DG],
                        wqT[:, c0 * DG : (c0 + nch) * DG],
                    )
                    nc.sync.dma_start(
                        wk_sb[:, c0 * DG : (c0 + nch) * DG],
                        wkT[:, c0 * DG : (c0 + nch) * DG],
                    )
                # xT by (c-quarter, s-half): 2KB lines, first-chunk columns
                # arrive c-by-c so QK(0) pipelines with the load
                xt3 = xt[:].rearrange("p (c s) -> p c s", c=NTC)
                xT3 = xT[:].rearrange("p (c s) -> p c s", c=NTC)
                for sh in range(2):
                    sl = slice(sh * 1024, (sh + 1) * 1024)
                    for c0, nch in quarters:
                        nc.gpsimd.dma_start(
                            xt3[:, c0 : c0 + nch, sl],
                            xT3[:, c0 : c0 + nch, sl],
                        )
                nc.gpsimd.dma_start(wv_sb[:], wvT[:])
                nc.scalar.dma_start(mask_t[:], mask[:])

                with tc.tile_pool(name="psqk", bufs=8, space="PSUM") as pqk:
                    for ic in range(NSC):
                        s0 = ic * 512
                        qps = [pqk.tile([128, 512], F32, tag="qk", name="qkps") for _ in range(HPG)]
                        kps = [pqk.tile([128, 512], F32, tag="qk", name="qkps") for _ in range(HPG)]
                        for c in range(NTC):
                            xs = xt[:, c * S + s0 : c * S + s0 + 512]
                            st, sp = c == 0, c == NTC - 1
                            for h in range(HPG):
                                nc.tensor.matmul(
                                    qps[h][:],
                                    wq_sb[:, c * DG + h * 128 : c * DG + (h + 1) * 128],
                                    xs, start=st, stop=sp,
                                )
                            for h in range(HPG):
                                nc.tensor.matmul(
                                    kps[h][:],
                                    wk_sb[:, c * DG + h * 128 : c * DG + (h + 1) * 128],
                                    xs, start=st, stop=sp,
                                )
                        for h in range(HPG):
                            nc.scalar.copy(qt[h][:, s0 : s0 + 512], qps[h][:])
                        for h in range(HPG):
                            nc.vector.tensor_copy(kt[h][:, s0 : s0 + 512], kps[h][:])

                    # ---- V phase, reusing the same PSUM slot ring ----
                    for ic in range(NSC):
                        s0 = ic * 512
                        vps = [pqk.tile([128, DG], F32, tag="qk", name="vps") for _ in range(4)]
                        for c in range(NTC):
                            st, sp = c == 0, c == NTC - 1
                            for stile in range(4):
                                nc.tensor.matmul(
                                    vps[stile][:],
                                    xt[:, c * S + s0 + stile * 128 : c * S + s0 + (stile + 1) * 128],
                                    wv_sb[:, c * DG : (c + 1) * DG],
                                    start=st, stop=sp,
                                )
                        for stile in range(4):
                            jj = ic * 4 + stile
                            if stile % 2 == 0:
                                nc.vector.tensor_copy(
                                    vt[:, jj * DG : (jj + 1) * DG], vps[stile][:]
                                )
                            else:
                                nc.scalar.copy(
                                    vt[:, jj * DG : (jj + 1) * DG], vps[stile][:]
                                )

            # ---- attention + out-projection ----
            if True:
                wo_sb = res.tile([128, HPG * D], BF16, tag="wo", name="wo")
                nc.scalar.dma_start(wo_sb[:], woT[:])
                etp = res
                with (
                    tc.tile_pool(name="psatt", bufs=8, space="PSUM") as patt,
                ):
                    for ic in range(NSC):
                        s0 = ic * 512
                        njj = (ic + 1) * 4
                        for h in range(HPG):
                            ctx_ps = patt.tile([128, 512], F32, tag="ctx", name="ctxps", bufs=3)
                            rs_ps = patt.tile([128, 512], F32, tag="rs", name="rsps", bufs=2)
                            for jj in range(njj):
                                j0 = jj * 128
                                rel = max(0, j0 - s0)
                                stp = patt.tile([128, 512], F32, tag="stp", name="stp", bufs=3)
                                et = etp.tile([128, 512], BF16, tag="et", name="et", bufs=4)
                                nc.tensor.matmul(
                                    stp[:, rel:512],
                                    kt[h][:, j0 : j0 + 128],
                                    qt[h][:, s0 + rel : s0 + 512],
                                    start=True, stop=True,
                                )
                                nc.scalar.activation(
                                    et[:, rel:512], stp[:, rel:512],
                                    EXP, bias=0.0, scale=SCALE,
                                )
                                if jj >= ic * 4:
                                    # diagonal tile: zero the k > q triangle
                                    nc.vector.tensor_mul(
                                        et[:, rel : rel + 128],
                                        et[:, rel : rel + 128],
                                        mask_t[:],
                                    )
                                nc.tensor.matmul(
                                    ctx_ps[:, rel:512],
                                    vt[:, jj * DG + h * 128 : jj * DG + (h + 1) * 128],
                                    et[:, rel:512],
                                    start=(jj == 0), stop=(jj == njj - 1),
                                )
                                nc.tensor.matmul(
                                    rs_ps[:, rel:512],
                                    onesb[:],
                                    et[:, rel:512],
                                    start=(jj == 0), stop=(jj == njj - 1),
                                )
                            rrb = etp.tile([128, 512], F32, tag="rrb", name="rrb", bufs=1)
                            nc.vector.reciprocal(rrb[:], rs_ps[:])
                            nc.vector.tensor_mul(
                                ct[h][:, s0 : s0 + 512], ctx_ps[:], rrb[:]
                            )
                        # out-projection for the previous q chunk: PE filler
                        # while attention here is ACT-bound
                        if not debug:
                            if ic > 0:
                                _outproj(nc, patt, etp, ct, wo_sb, y, ic - 1)
                    if not debug:
                        _outproj(nc, patt, etp, ct, wo_sb, y, NSC - 1)
                    elif debug == 1:
                        nc.sync.dma_start(y[0:128, :], qt[0][:])
                        nc.sync.dma_start(y[128:256, :], kt[0][:])
                        nc.sync.dma_start(y[256:384, :], vt[:, 0:2048])
                        nc.sync.dma_start(y[384:512, :], ct[0][:])
                    else:
                        for h in range(HPG):
                            nc.sync.dma_start(y[h * 128 : (h + 1) * 128, :], ct[h][:])
    nc.finalize()
    return nc


def get_nc():
    if "nc" not in _CACHE:
        _CACHE["nc"] = _build()
    return _CACHE["nc"]


def make_in_maps(inputs, w_q, w_k, w_v, w_o, b_o):
    bf = ml_dtypes.bfloat16
    x = np.asarray(inputs, dtype=np.float32)
    w_q = np.asarray(w_q, dtype=np.float32)
    w_k = np.asarray(w_k, dtype=np.float32)
    w_v = np.asarray(w_v, dtype=np.float32)
    w_o = np.asarray(w_o, dtype=np.float32)

    # causal 0/1 mask in [k, q] orientation: keep k <= q
    idx = np.arange(128)
    mask = (idx[:, None] <= idx[None, :]).astype(np.float32)

    def packT(a):
        # [D, cols] -> [128, NTC*cols]: SBUF layout, partition = row % 128
        cols = a.shape[1]
        return np.ascontiguousarray(
            a.reshape(NTC, 128, cols).transpose(1, 0, 2).reshape(128, NTC * cols)
        ).astype(bf)

    xTs = [packT(x[b].T) for b in range(B)]
    wqTs = [packT(w_q[g * DG : (g + 1) * DG, :].T) for g in range(G)]
    wkTs = [packT(w_k[g * DG : (g + 1) * DG, :].T) for g in range(G)]
    wvTs = [packT(w_v[g * DG : (g + 1) * DG, :].T) for g in range(G)]
    woTs = [
        np.ascontiguousarray(
            w_o[:, g * DG : (g + 1) * DG].T.reshape(HPG, 128, D)
            .transpose(1, 0, 2).reshape(128, HPG * D)
        ).astype(bf)
        for g in range(G)
    ]

    in_maps = []
    for core in range(NCORES):
        b, g = divmod(core, G)
        in_maps.append(
            {
                "xT": xTs[b],
                "wqT": wqTs[g],
                "wkT": wkTs[g],
                "wvT": wvTs[g],
                "woT": woTs[g],
                "mask": mask,
            }
        )
    return in_maps


def assemble(results, b_o):
    out = np.zeros((B, S, D), dtype=np.float32)
    for core in range(NCORES):
        b = core // G
        out[b] += np.asarray(results[core]["y"], dtype=np.float32)
    out += np.asarray(b_o, dtype=np.float32)[None, None, :]
    return out


def kernel(inputs, w_q, w_k, w_v, w_o, b_o):
    nc = get_nc()
    in_maps = make_in_maps(inputs, w_q, w_k, w_v, w_o, b_o)
    res = run_bass_kernel_spmd(nc, in_maps, core_ids=list(range(NCORES)))
    return assemble(res.results, b_o)
